# revision 1
# baseline (speedup 1.0000x reference)
"""Trainium2 Bass kernel for CLDOdeBlock (graph ODE, RK4 over batch-sharded cores).

Math (per batch b):
    An = adjacency / max(adjacency.sum(-1, keepdims=True), 1)
    vector_field(t, h) = tanh([h | An@h | te(t)] @ W1 + b1) @ W2 + b2
    RK4 with 8 steps over time_grid; output trajectory [B, T, C, D].

Mapping to the PE (out = lhsT.T @ rhs, contraction on partitions):
  - aggT = matmul(lhsT=h, rhs=AnT)         -> agg transposed [d, row], both
    operands in their natural SBUF layouts (AnT is host-pre-transposed).
  - out1T = matmul(lhsT=W1[:512] chunks, rhs=[hT | aggT]) with the Fourier
    time-embedding columns of W1 folded into a per-eval bias on the host:
    b1_eff = b1 + te(t) @ W1[512:544].
  - tanh fused into the PSUM->SBUF evacuation on the scalar engine.
  - out2 (natural) = matmul(lhsT=a1T chunks, rhs=W2); bias b2 fused in the
    DVE evacuation.
  - hT produced with 16 PE transposes per stage per batch.
Degree normalization is folded into AnT on the host (pure input prep).

float32r (TF32) is used for all matmul inputs: the hardware requires every
producer of an FP32r matmul operand to emit FP32r-rounded output, so host
inputs are pre-rounded to TF32 and on-device producers (ACT evacuations,
DVE state updates) write through f32r-typed output APs.
"""

import math
from contextlib import ExitStack, nullcontext

import numpy as np

import concourse.bass as bass
import concourse.tile as tile
from concourse import bacc, mybir
from concourse.bass import ds

B, C, D = 16, 1024, 256
T = 9
NSTEP_FULL = T - 1
NCORES = 8
BPC = B // NCORES  # batches per core
TIME_DIM = 32
HALF = TIME_DIM // 2
F32 = mybir.dt.float32
F32R = mybir.dt.float32r

RT = C // 128   # 8 row tiles
DT = D // 128   # 2 feature tiles
NH = C // 512   # 2 free halves for N=512 matmuls


def build_program(dts, n_steps=NSTEP_FULL, n_iters=1, use_f32r=True,
                  skip_stores=False, skip_transpose=False):
    """Build + compile the per-core Bass program.

    dts: python floats, len n_steps (the RK4 dt per step; baked in).
    n_iters: >1 wraps the whole computation in a For_i loop (for timing).
    """
    nc = bacc.Bacc("TRN2", target_bir_lowering=False, debug=False)

    at_d = nc.dram_tensor("at", [BPC, RT, 128, C], F32, kind="ExternalInput").ap()
    h0_d = nc.dram_tensor("h0", [BPC, 128, RT, D], F32, kind="ExternalInput").ap()
    w1_d = nc.dram_tensor("w1", [128, 4, DT, 128], F32, kind="ExternalInput").ap()
    w2_d = nc.dram_tensor("w2", [128, DT, D], F32, kind="ExternalInput").ap()
    b1_d = nc.dram_tensor("b1t", [128, DT, 4 * NSTEP_FULL], F32, kind="ExternalInput").ap()
    b2_d = nc.dram_tensor("b2r", [128, D], F32, kind="ExternalInput").ap()
    id_d = nc.dram_tensor("ident", [128, 128], F32, kind="ExternalInput").ap()
    tr_d = nc.dram_tensor("traj", [BPC, n_steps, 128, RT, D], F32, kind="ExternalOutput").ap()

    def mm(ap):
        return ap.bitcast(F32R) if use_f32r else ap

    with ExitStack() as ctx:
        tc = ctx.enter_context(tile.TileContext(nc))
        const = ctx.enter_context(tc.tile_pool(name="const", bufs=1))
        at_p = ctx.enter_context(tc.tile_pool(name="atp", bufs=1))

        # ---- constants / weights (matmul-feeding tensors move as f32r) ----
        at_sb = at_p.tile([128, BPC, RT, C], F32)
        for b in range(BPC):
            for jc in range(RT):
                nc.sync.dma_start(mm(at_sb[:, b, jc, :]), mm(at_d[b, jc]))
        w1_sb = const.tile([128, 4, DT, 128], F32)
        nc.sync.dma_start(mm(w1_sb[:]), mm(w1_d))
        w2_sb = const.tile([128, DT, D], F32)
        nc.sync.dma_start(mm(w2_sb[:]), mm(w2_d))
        b1_sb = const.tile([128, DT, 4 * NSTEP_FULL], F32)
        nc.sync.dma_start(b1_sb[:], b1_d)
        b2_sb = const.tile([128, D], F32)
        nc.sync.dma_start(b2_sb[:], b2_d)
        id_sb = const.tile([128, 128], F32)
        nc.sync.dma_start(mm(id_sb[:]), mm(id_d))

        # b2 broadcast along the row-tile axis: [128, RT, D] view of [128, D]
        b2_ap = b2_sb[:]
        b2_bc = bass.AP(
            tensor=b2_ap.tensor,
            offset=b2_ap.offset,
            ap=[b2_ap.ap[0], [0, RT], b2_ap.ap[1]],
        )

        # ---- main pools ----
        state_p = ctx.enter_context(tc.tile_pool(name="state", bufs=4))
        hs_p = ctx.enter_context(tc.tile_pool(name="hs", bufs=2))
        k_p = ctx.enter_context(tc.tile_pool(name="k", bufs=2))
        acc_p = ctx.enter_context(tc.tile_pool(name="acc", bufs=2))
        tp_p = ctx.enter_context(tc.tile_pool(name="tp", bufs=4))
        ps_p = ctx.enter_context(tc.tile_pool(name="ps", bufs=2, space="PSUM"))

        loop_cm = tc.For_i(0, n_iters) if n_iters > 1 else nullcontext()
        with loop_cm:
            hstates = []
            for b in range(BPC):
                hst = state_p.tile([128, RT, D], F32, tag="hst")
                nc.sync.dma_start(mm(hst[:]), mm(h0_d[b]))
                hstates.append(hst)
            hstage = [None] * BPC
            acc = [None] * BPC

            for s in range(n_steps):
                dt = float(dts[s])
                for g in range(4):
                    ev = s * 4 + g
                    for b in range(BPC):
                        h_in = hstates[b] if g == 0 else hstage[b]

                        # 1) hT = h.T via f32r transpose-mode, per-half evacs so the
                        # ACT copies overlap the agg matmuls; 2) aggT = (An@h).T
                        # with d_=0's evac hidden under d_=1's matmuls.
                        pt = ps_p.tile([128, DT, C], F32, tag="ps")
                        pa = ps_p.tile([128, DT, C], F32, tag="ps")
                        hT = tp_p.tile([128, DT, C], F32, tag="tp")
                        agT = tp_p.tile([128, DT, C], F32, tag="tp")
                        for d_ in range(DT):
                            for jc in range(RT):
                                nc.tensor.transpose(
                                    mm(pt[:, d_, ds(jc * 128, 128)]),
                                    mm(h_in[:, jc, ds(d_ * 128, 128)]),
                                    mm(id_sb[:]),
                                )
                            nc.scalar.copy(mm(hT[:, d_, :]), pt[:, d_, :])
                        for d_ in range(DT):
                            for jc in range(RT):
                                for nh in range(NH):
                                    nc.tensor.matmul(
                                        pa[:, d_, ds(nh * 512, 512)],
                                        mm(h_in[:, jc, ds(d_ * 128, 128)]),
                                        mm(at_sb[:, b, jc, ds(nh * 512, 512)]),
                                        start=(jc == 0),
                                        stop=(jc == RT - 1),
                                    )
                            nc.scalar.copy(mm(agT[:, d_, :]), pa[:, d_, :])

                        # 3) out1T = W1h.T @ hT + W1a.T @ aggT; tanh(+b1_eff) fused
                        p1 = ps_p.tile([128, DT, C], F32, tag="ps")
                        for ht in range(DT):
                            for kc in range(4):
                                src = (hT if not skip_transpose else agT) if kc < 2 else agT
                                kcs = kc % 2
                                for nh in range(NH):
                                    nc.tensor.matmul(
                                        p1[:, ht, ds(nh * 512, 512)],
                                        mm(w1_sb[:, kc, ht, :]),
                                        mm(src[:, kcs, ds(nh * 512, 512)]),
                                        start=(kc == 0),
                                        stop=(kc == 3),
                                    )
                        a1 = tp_p.tile([128, DT, C], F32, tag="tp")
                        for ht in range(DT):
                            nc.scalar.activation(
                                mm(a1[:, ht, :]),
                                p1[:, ht, :],
                                mybir.ActivationFunctionType.Tanh,
                                bias=b1_sb[:, ht, ev : ev + 1],
                                scale=1.0,
                            )

                        # 4) out2 = a1 @ W2 (natural layout) [128, RT, D]
                        p2 = ps_p.tile([128, RT, D], F32, tag="ps")
                        for rt in range(RT):
                            for kc in range(DT):
                                nc.tensor.matmul(
                                    p2[:, rt, :],
                                    mm(a1[:, kc, ds(rt * 128, 128)]),
                                    mm(w2_sb[:, kc, :]),
                                    start=(kc == 0),
                                    stop=(kc == DT - 1),
                                )

                        # 5) k = out2 + b2
                        k = k_p.tile([128, RT, D], F32, tag="k")
                        nc.vector.tensor_add(k[:], p2[:], b2_bc)

                        # 6) RK4 state updates
                        if g == 0:
                            a = acc_p.tile([128, RT, D], F32, tag="acc")
                            nc.vector.tensor_copy(a[:], k[:])
                            acc[b] = a
                        else:
                            w = 2.0 if g < 3 else 1.0
                            nc.vector.scalar_tensor_tensor(
                                acc[b][:], k[:], w, acc[b][:],
                                mybir.AluOpType.mult, mybir.AluOpType.add,
                            )
                        if g < 3:
                            c = dt / 2 if g < 2 else dt
                            hs = hs_p.tile([128, RT, D], F32, tag="hs")
                            nc.vector.scalar_tensor_tensor(
                                mm(hs[:]), k[:], c, hstates[b][:],
                                mybir.AluOpType.mult, mybir.AluOpType.add,
                            )
                            hstage[b] = hs
                        else:
                            hn = state_p.tile([128, RT, D], F32, tag="hst")
                            nc.vector.scalar_tensor_tensor(
                                mm(hn[:]), acc[b][:], dt / 6.0, hstates[b][:],
                                mybir.AluOpType.mult, mybir.AluOpType.add,
                            )
                            hstates[b] = hn
                            if not skip_stores:
                                nc.sync.dma_start(tr_d[b, s], hn[:])

    nc.compile()
    return nc


def tf32_round(x):
    """Round-to-nearest-even to TF32 (10 mantissa bits) — what the PE's
    FP32r mode expects its operands to already be."""
    u = np.ascontiguousarray(x, np.float32).view(np.uint32)
    lsb = (u >> np.uint32(13)) & np.uint32(1)
    u = u + np.uint32(0x0FFF) + lsb
    u &= np.uint32(0xFFFFE000)
    return u.view(np.float32)


def host_prep(h0, time_grid, adjacency, W1, b1, W2, b2, n_steps=NSTEP_FULL,
              use_f32r=True):
    """Returns (in_maps list per core, dts list)."""
    h0 = np.asarray(h0, np.float32)
    time_grid = np.asarray(time_grid, np.float32)
    adjacency = np.asarray(adjacency, np.float32)
    W1 = np.asarray(W1, np.float32)
    b1 = np.asarray(b1, np.float32)
    W2 = np.asarray(W2, np.float32)
    b2 = np.asarray(b2, np.float32)

    # degree normalization (time-constant input prep)
    deg = np.maximum(adjacency.sum(-1, keepdims=True), np.float32(1.0))
    adjacency = (adjacency / deg).astype(np.float32)

    rnd = tf32_round if use_f32r else (lambda x: x)

    # Fourier features folded into per-eval bias
    freqs = np.exp(
        -math.log(10000.0) * np.arange(HALF, dtype=np.float32) / np.float32(HALF)
    ).astype(np.float32)

    def te(t):
        a = (np.float32(t) * freqs).astype(np.float32)
        return np.concatenate([np.sin(a), np.cos(a)]).astype(np.float32)

    dts = []
    b1_eff = np.zeros((4 * NSTEP_FULL, D), np.float32)
    for s in range(NSTEP_FULL):
        t0 = np.float32(time_grid[s])
        t1 = np.float32(time_grid[s + 1])
        dt = np.float32(t1 - t0)
        dts.append(float(dt))
        stage_ts = [t0, np.float32(t0 + dt / 2), np.float32(t0 + dt / 2),
                    np.float32(t0 + dt)]
        for g, tg_ in enumerate(stage_ts):
            b1_eff[s * 4 + g] = b1 + te(tg_) @ W1[2 * D:]

    w1_in = rnd(np.ascontiguousarray(
        W1[: 2 * D].reshape(4, 128, DT, 128).transpose(1, 0, 2, 3)
    ))
    w2_in = rnd(np.ascontiguousarray(W2.reshape(DT, 128, D).transpose(1, 0, 2)))
    b1_in = np.ascontiguousarray(
        b1_eff.reshape(4 * NSTEP_FULL, DT, 128).transpose(2, 1, 0)
    )
    b2_in = np.ascontiguousarray(np.tile(b2[None, :], (128, 1)))
    ident = np.eye(128, dtype=np.float32)

    in_maps = []
    for ci in range(NCORES):
        sl = slice(ci * BPC, (ci + 1) * BPC)
        at_in = rnd(np.ascontiguousarray(
            adjacency[sl].transpose(0, 2, 1).reshape(BPC, RT, 128, C)
        ))
        h0_in = rnd(np.ascontiguousarray(
            h0[sl].reshape(BPC, RT, 128, D).transpose(0, 2, 1, 3)))
        in_maps.append(
            {
                "at": at_in,
                "h0": h0_in,
                "w1": w1_in,
                "w2": w2_in,
                "b1t": b1_in,
                "b2r": b2_in,
                "ident": ident,
            }
        )
    return in_maps, dts[:n_steps]


def gather(results, h0, n_steps=NSTEP_FULL):
    h0 = np.asarray(h0, np.float32)
    out = np.empty((B, n_steps + 1, C, D), np.float32)
    out[:, 0] = h0
    for ci in range(NCORES):
        t = results[ci]["traj"]  # [BPC, n_steps, 128, RT, D]
        out[ci * BPC : (ci + 1) * BPC, 1:] = t.transpose(0, 1, 3, 2, 4).reshape(
            BPC, n_steps, C, D
        )
    return out


_CACHE = {}


def kernel(h0, time_grid, adjacency, W1, b1, W2, b2):
    from concourse.bass_utils import run_bass_kernel_spmd

    in_maps, dts = host_prep(h0, time_grid, adjacency, W1, b1, W2, b2)
    key = tuple(dts)
    if key not in _CACHE:
        _CACHE[key] = build_program(dts)
    nc = _CACHE[key]
    res = run_bass_kernel_spmd(nc, in_maps, list(range(NCORES)), trace=False)
    return gather(res.results, h0)

